# revision 20
# baseline (speedup 1.0000x reference)
"""Trainium2 Bass kernel for nn_PlanMapBoundLoss.

Strategy (pure data parallel, one batch per SBUF partition):
  - 8 cores x 128 batches each (B=1024).
  - Per core: lane polylines live as [128, 4000] f32 in SBUF (xy interleaved).
  - Per timestep t (6): squared distances to all 2000 lane points are computed
    with two ScalarE Square activations (denorm fused in: Square(scale*x+bias)
    with per-partition bias = -offset - pred_t), summed on VectorE, then a
    grouped min-reduce (negated) gives per-lane -min dist^2 [128, 100].
    A -1e30 penalty masks non-boundary lanes; Max8 + MaxIndex give the
    nearest-lane id and -min dist^2.
  - The nearest lane's 20 points are fetched per-batch with an indirect DMA
    gather from HBM (row = (b, v), 160B).
  - Segment intersection (ego segment t vs 19 lane segments) is evaluated
    division-free on VectorE at [128, 6, 19]; its cummax over t zeroes the
    loss suffix. Per-batch losses [128, 6] are DMA'd out; the host sums.
"""

import sys

if "/opt/trn_rl_repo" not in sys.path:
    sys.path.insert(0, "/opt/trn_rl_repo")

from contextlib import ExitStack

import numpy as np

import concourse.bacc as bacc
import concourse.bass as bass
import concourse.tile as tile
from concourse import mybir
from concourse.bass import AP

F32 = mybir.dt.float32
U32 = mybir.dt.uint32
Alu = mybir.AluOpType
Act = mybir.ActivationFunctionType
Axis = mybir.AxisListType

B, T, V, P, C = 1024, 6, 100, 20, 3
NCORES = 8
BL = B // NCORES  # 128 batches per core == partition count
J = V * P  # 2000 lane points per batch
SCALE_X, OFF_X = 30.0, -15.0  # pc_range denorm
SCALE_Y, OFF_Y = 60.0, -30.0
MAP_THRESH = 0.5
DIS_THRESH = 1.0
PEN = -1.0e30

NJCHUNK = 2  # split the 2000-wide work for DMA/compute overlap
JC = J // NJCHUNK  # 1000 points per chunk
VC = V // NJCHUNK  # 50 lanes per chunk


def _bcast_last(ap: AP, n: int) -> AP:
    """Append a stride-0 broadcast dim of size n to an AP."""
    return AP(tensor=ap.tensor, offset=ap.offset, ap=ap.ap + [[0, n]])


def build_nc(debug_taps=False):
    nc = bacc.Bacc(None, target_bir_lowering=False)

    lane = nc.dram_tensor("lane", [BL, V, P, 2], F32, kind="ExternalInput")
    score = nc.dram_tensor("score", [BL, V, C], F32, kind="ExternalInput")
    ego = nc.dram_tensor("ego", [BL, T, 2], F32, kind="ExternalInput")
    out = nc.dram_tensor("loss", [BL, T], F32, kind="ExternalOutput")

    lane_flat = lane[:].rearrange("b v p c -> b (v p c)")  # [128, 4000]
    lane_rows = lane[:].rearrange("b v p c -> (b v) (p c)")  # [12800, 40]

    with ExitStack() as ctx:
        tc = ctx.enter_context(tile.TileContext(nc))
        singles = ctx.enter_context(tc.tile_pool(name="singles", bufs=1))
        work = ctx.enter_context(tc.tile_pool(name="work", bufs=3))

        # ---- loads: small tensors on the gpsimd queue, lane bulk on sync ----
        score_sb = singles.tile([BL, V * C], F32)
        nc.gpsimd.dma_start(out=score_sb[:], in_=score[:].rearrange("b v c -> b (v c)"))
        ego_sb = singles.tile([BL, T * 2], F32)
        nc.gpsimd.dma_start(out=ego_sb[:], in_=ego[:].rearrange("b t c -> b (t c)"))
        lane_sb = singles.tile([BL, 2 * J], F32)  # xy interleaved
        for c in range(NJCHUNK):
            nc.sync.dma_start(
                out=lane_sb[:, c * 2 * JC : (c + 1) * 2 * JC],
                in_=lane_flat[:, c * 2 * JC : (c + 1) * 2 * JC],
            )
        # dummy Square: hoist the ACT table load off the critical path
        dumm = singles.tile([BL, 8], F32)
        nc.vector.memset(dumm[:], 0.0)
        nc.scalar.activation(out=dumm[:], in_=dumm[:], func=Act.Square, bias=0.0, scale=1.0)

        lane3 = lane_sb[:].rearrange("b (j c) -> b j c", c=2)
        lane_x = lane3[:, :, 0]  # [128, 2000] stride-2 view
        lane_y = lane3[:, :, 1]

        # ---- penalty for non-boundary lanes ----
        pen = singles.tile([BL, V], F32)
        score3 = score_sb[:].rearrange("b (v c) -> b v c", c=C)
        nc.vector.tensor_scalar(
            out=pen[:],
            in0=score3[:, :, 2],
            scalar1=MAP_THRESH,
            scalar2=PEN,
            op0=Alu.is_lt,
            op1=Alu.mult,
        )

        # ---- trajectory cumsum and per-t scalars ----
        ego3 = ego_sb[:].rearrange("b (t c) -> b t c", c=2)
        ego_x = ego3[:, :, 0]  # [128, 6] stride-2
        ego_y = ego3[:, :, 1]
        zeros6 = singles.tile([BL, T], F32)
        nc.vector.memset(zeros6[:], 0.0)
        px = singles.tile([BL, T], F32)
        py = singles.tile([BL, T], F32)
        nc.vector.tensor_tensor_scan(
            out=px[:], data0=ego_x, data1=zeros6[:], initial=0.0,
            op0=Alu.add, op1=Alu.add,
        )
        nc.vector.tensor_tensor_scan(
            out=py[:], data0=ego_y, data1=zeros6[:], initial=0.0,
            op0=Alu.add, op1=Alu.add,
        )
        # biases for the Square activation: -OFF - pred
        biasx = singles.tile([BL, T], F32)
        biasy = singles.tile([BL, T], F32)
        nc.vector.tensor_scalar(
            out=biasx[:], in0=px[:], scalar1=-1.0, scalar2=OFF_X,
            op0=Alu.mult, op1=Alu.add,
        )
        nc.vector.tensor_scalar(
            out=biasy[:], in0=py[:], scalar1=-1.0, scalar2=OFF_Y,
            op0=Alu.mult, op1=Alu.add,
        )
        # segment starts: s1[t] = pred[t-1], s1[0] = 0
        s1x = singles.tile([BL, T], F32)
        s1y = singles.tile([BL, T], F32)
        nc.vector.memset(s1x[:, 0:1], 0.0)
        nc.vector.memset(s1y[:, 0:1], 0.0)
        nc.vector.tensor_copy(s1x[:, 1:T], px[:, 0 : T - 1])
        nc.vector.tensor_copy(s1y[:, 1:T], py[:, 0 : T - 1])
        # d1[t] = pred[t] - s1[t] (match reference rounding)
        d1x = singles.tile([BL, T], F32)
        d1y = singles.tile([BL, T], F32)
        nc.vector.tensor_tensor(out=d1x[:], in0=px[:], in1=s1x[:], op=Alu.subtract)
        nc.vector.tensor_tensor(out=d1y[:], in0=py[:], in1=s1y[:], op=Alu.subtract)

        # row index base for the gather: 100*b
        iotap = singles.tile([BL, 1], U32)
        nc.gpsimd.iota(iotap[:], pattern=[[0, 1]], base=0, channel_multiplier=V)

        # ---- persistent per-t outputs ----
        top8all = singles.tile([BL, T * 8], F32)
        idx8all = singles.tile([BL, T * 8], U32)
        rowidx = singles.tile([BL, T], U32)
        TH = T // 2
        # two gather tiles so the first intersection half only waits on t0-2
        gathA = singles.tile([BL, TH * P * 2], F32)
        gathB = singles.tile([BL, TH * P * 2], F32)

        # ---- main t loop ----
        for t in range(T):
            sqx = work.tile([BL, J], F32, tag="sqx")
            sqy = work.tile([BL, J], F32, tag="sqy")
            ss = work.tile([BL, J], F32, tag="ss")
            dmn = work.tile([BL, V], F32, tag="dmn")
            dm2 = work.tile([BL, V], F32, tag="dm2")
            if t == 0:
                # chunked: lets the first squares start on the first DMA chunk
                for c in range(NJCHUNK):
                    js = slice(c * JC, (c + 1) * JC)
                    nc.scalar.activation(
                        out=sqx[:, js], in_=lane_x[:, js], func=Act.Square,
                        bias=biasx[:, t : t + 1], scale=SCALE_X,
                    )
                    nc.scalar.activation(
                        out=sqy[:, js], in_=lane_y[:, js], func=Act.Square,
                        bias=biasy[:, t : t + 1], scale=SCALE_Y,
                    )
            else:
                nc.scalar.activation(
                    out=sqx[:], in_=lane_x, func=Act.Square,
                    bias=biasx[:, t : t + 1], scale=SCALE_X,
                )
                nc.scalar.activation(
                    out=sqy[:], in_=lane_y, func=Act.Square,
                    bias=biasy[:, t : t + 1], scale=SCALE_Y,
                )
            # split the adds: first half on DVE, second half on GpSimd
            nc.vector.tensor_tensor(
                out=ss[:, 0:JC], in0=sqx[:, 0:JC], in1=sqy[:, 0:JC], op=Alu.add
            )
            nc.gpsimd.tensor_tensor(
                out=ss[:, JC:J], in0=sqx[:, JC:J], in1=sqy[:, JC:J], op=Alu.add
            )
            ss3 = ss[:].rearrange("b (v p) -> b v p", p=P)
            nc.vector.tensor_reduce(
                out=dmn[:], in_=ss3, op=Alu.min, axis=Axis.X, negate=True,
            )
            nc.vector.tensor_tensor(out=dm2[:], in0=dmn[:], in1=pen[:], op=Alu.add)
            t8 = top8all[:, 8 * t : 8 * t + 8]
            i8 = idx8all[:, 8 * t : 8 * t + 8]
            nc.vector.max(t8, dm2[:])
            nc.vector.max_index(i8, t8, dm2[:])
            nc.vector.tensor_tensor(
                out=rowidx[:, t : t + 1], in0=iotap[:], in1=i8[:, 0:1], op=Alu.add
            )
            gtile = gathA if t < TH else gathB
            toff = t if t < TH else t - TH
            nc.gpsimd.indirect_dma_start(
                out=gtile[:, 2 * P * toff : 2 * P * (toff + 1)],
                out_offset=None,
                in_=lane_rows,
                in_offset=bass.IndirectOffsetOnAxis(ap=rowidx[:, t : t + 1], axis=0),
            )
            if debug_taps and t == 0:
                for nm, tl in (("d_sqx", sqx), ("d_sqy", sqy), ("d_ss", ss), ("d_dmn", dmn), ("d_dm2", dm2)):
                    dbg = nc.dram_tensor(nm, list(tl.shape), F32, kind="ExternalOutput")
                    nc.sync.dma_start(out=dbg[:], in_=tl[:])

        # ---- min distance -> loss ----
        t8v = top8all[:].rearrange("b (t e) -> b t e", e=8)
        mind = singles.tile([BL, T], F32)
        nc.scalar.activation(out=mind[:], in_=t8v[:, :, 0], func=Act.Sqrt, scale=-1.0)
        loss6 = singles.tile([BL, T], F32)
        nc.scalar.activation(
            out=loss6[:], in_=mind[:], func=Act.Relu, scale=-1.0, bias=DIS_THRESH
        )

        # ---- segment intersection, two t-halves (first overlaps later gathers) ----
        S = P - 1  # 19 segments
        inter = singles.tile([BL, T], F32)

        for h in range(2):
            ts0 = h * TH
            tsl = slice(ts0, ts0 + TH)
            gsrc = gathA if h == 0 else gathB
            g3 = gsrc[:].rearrange("b (t p c) -> b t p c", p=P, c=2)
            gx = work.tile([BL, TH * P], F32, tag=f"gx{h}", name=f"gx{h}")
            gy = work.tile([BL, TH * P], F32, tag=f"gy{h}", name=f"gy{h}")
            gx3 = gx[:].rearrange("b (t p) -> b t p", p=P)
            gy3 = gy[:].rearrange("b (t p) -> b t p", p=P)
            nc.scalar.activation(
                out=gx3, in_=g3[:, :, :, 0], func=Act.Copy, scale=SCALE_X, bias=OFF_X
            )
            nc.scalar.activation(
                out=gy3, in_=g3[:, :, :, 1], func=Act.Copy, scale=SCALE_Y, bias=OFF_Y
            )
            W = TH * S

            def wt(name_tag):
                tl = work.tile([BL, W], F32, tag=name_tag, name=name_tag)
                return tl[:].rearrange("b (t s) -> b t s", s=S)

            s2x = gx3[:, :, 0:S]
            s2y = gy3[:, :, 0:S]
            e2x = gx3[:, :, 1 : S + 1]
            e2y = gy3[:, :, 1 : S + 1]
            d2x = wt(f"d2x{h}")
            d2y = wt(f"d2y{h}")
            nc.vector.tensor_tensor(out=d2x, in0=e2x, in1=s2x, op=Alu.subtract)
            nc.vector.tensor_tensor(out=d2y, in0=e2y, in1=s2y, op=Alu.subtract)
            dlx = wt(f"dlx{h}")
            dly = wt(f"dly{h}")
            nc.vector.tensor_tensor(
                out=dlx, in0=s2x, in1=_bcast_last(s1x[:, tsl], S), op=Alu.subtract
            )
            nc.vector.tensor_tensor(
                out=dly, in0=s2y, in1=_bcast_last(s1y[:, tsl], S), op=Alu.subtract
            )
            # det = d1x*d2y - d1y*d2x
            ta = wt(f"ta{h}")
            tb = wt(f"tb{h}")
            det = wt(f"det{h}")
            nc.vector.tensor_tensor(out=ta, in0=d2y, in1=_bcast_last(d1x[:, tsl], S), op=Alu.mult)
            nc.vector.tensor_tensor(out=tb, in0=d2x, in1=_bcast_last(d1y[:, tsl], S), op=Alu.mult)
            nc.vector.tensor_tensor(out=det, in0=ta, in1=tb, op=Alu.subtract)
            # t_num = dlx*d2y - dly*d2x
            tnum = wt(f"tnum{h}")
            nc.vector.tensor_tensor(out=ta, in0=dlx, in1=d2y, op=Alu.mult)
            nc.vector.tensor_tensor(out=tb, in0=dly, in1=d2x, op=Alu.mult)
            nc.vector.tensor_tensor(out=tnum, in0=ta, in1=tb, op=Alu.subtract)
            # u_num = dlx*d1y - dly*d1x  (u-chain on GpSimd, parallel with t-chain)
            unum = wt(f"unum{h}")
            ua = wt(f"ua{h}")
            ub = wt(f"ub{h}")
            nc.gpsimd.tensor_tensor(
                out=ua, in0=dlx, in1=_bcast_last(d1y[:, tsl], S), op=Alu.mult
            )
            nc.gpsimd.tensor_tensor(
                out=ub, in0=dly, in1=_bcast_last(d1x[:, tsl], S), op=Alu.mult
            )
            nc.gpsimd.tensor_tensor(out=unum, in0=ua, in1=ub, op=Alu.subtract)
            # conditions: t in [0,1], u in [0,1], det != 0 (division-free, exact)
            cmin = wt(f"cmin{h}")
            nc.vector.tensor_tensor(out=ta, in0=tnum, in1=det, op=Alu.mult)  # t>=0
            nc.vector.tensor_tensor(out=tb, in0=det, in1=tnum, op=Alu.subtract)
            nc.vector.tensor_tensor(out=tb, in0=tb, in1=det, op=Alu.mult)  # t<=1
            nc.vector.tensor_tensor(out=cmin, in0=ta, in1=tb, op=Alu.min)
            nc.gpsimd.tensor_tensor(out=ua, in0=unum, in1=det, op=Alu.mult)  # u>=0
            nc.gpsimd.tensor_tensor(out=ub, in0=det, in1=unum, op=Alu.subtract)
            nc.gpsimd.tensor_tensor(out=ub, in0=ub, in1=det, op=Alu.mult)  # u<=1
            nc.gpsimd.tensor_tensor(out=ua, in0=ua, in1=ub, op=Alu.min)
            nc.vector.tensor_tensor(out=cmin, in0=cmin, in1=ua, op=Alu.min)
            # inter01 = (cmin >= 0) * (det != 0)
            nc.vector.tensor_scalar(
                out=cmin, in0=cmin, scalar1=0.0, scalar2=None, op0=Alu.is_ge
            )
            nc.vector.tensor_scalar(
                out=ta, in0=det, scalar1=0.0, scalar2=None, op0=Alu.not_equal
            )
            nc.vector.tensor_tensor(out=cmin, in0=cmin, in1=ta, op=Alu.mult)
            nc.vector.tensor_reduce(out=inter[:, tsl], in_=cmin, op=Alu.max, axis=Axis.X)

        # ---- zero the suffix from the first intersecting t ----
        cm = singles.tile([BL, T], F32)
        nc.vector.tensor_tensor_scan(
            out=cm[:], data0=inter[:], data1=zeros6[:], initial=0.0,
            op0=Alu.max, op1=Alu.add,
        )
        keep = singles.tile([BL, T], F32)
        nc.vector.tensor_scalar(
            out=keep[:], in0=cm[:], scalar1=-1.0, scalar2=1.0,
            op0=Alu.mult, op1=Alu.add,
        )
        lf = singles.tile([BL, T], F32)
        nc.vector.tensor_tensor(out=lf[:], in0=loss6[:], in1=keep[:], op=Alu.mult)
        nc.sync.dma_start(out=out[:], in_=lf[:])

        if debug_taps:
            taps = {
                "d_px": px, "d_biasx": biasx, "d_pen": pen, "d_top8": top8all,
                "d_idx8": idx8all, "d_rowidx": rowidx,
                "d_mind": mind, "d_loss6": loss6, "d_inter": inter,
                "d_cm": cm, "d_keep": keep,
            }
            for nm, tl in taps.items():
                dt_ = tl.tensor.dtype if hasattr(tl, "tensor") else F32
                dbg = nc.dram_tensor(nm, list(tl.shape), dt_, kind="ExternalOutput")
                nc.sync.dma_start(out=dbg[:], in_=tl[:])

    return nc


def _install_ntff_hook():
    """Provide antenv.axon_hooks (missing in this image) so trace=True works.

    Replicates trn_agent_boot._ntff_profile_via_ctypes against the axon PJRT
    .so: start/stop NRT profiling around the execute, NTFFs land in the
    requested output dir.
    """
    import sys as _sys
    import types
    import contextlib
    import ctypes

    if "antenv.axon_hooks" in _sys.modules:
        return
    try:
        import antenv
    except ImportError:
        return
    so_path = "/opt/axon/libaxon_pjrt.so"
    mod = types.ModuleType("antenv.axon_hooks")
    mod._hook = None

    def set_axon_ntff_profile_hook(h):
        mod._hook = h

    def get_axon_ntff_profile_hook():
        return mod._hook

    mod.set_axon_ntff_profile_hook = set_axon_ntff_profile_hook
    mod.get_axon_ntff_profile_hook = get_axon_ntff_profile_hook
    _sys.modules["antenv.axon_hooks"] = mod
    antenv.axon_hooks = mod

    try:
        lib = ctypes.CDLL(so_path)
    except OSError:
        return
    if not hasattr(lib, "axon_start_nrt_profile"):
        return
    lib.axon_start_nrt_profile.argtypes = [
        ctypes.POINTER(ctypes.c_int64),
        ctypes.c_size_t,
    ]
    lib.axon_start_nrt_profile.restype = ctypes.c_int64
    lib.axon_stop_nrt_profile.argtypes = [ctypes.c_char_p]
    lib.axon_stop_nrt_profile.restype = ctypes.c_int64

    @contextlib.contextmanager
    def _hook(output_dir, device_ids):
        import jax

        jax.devices()
        if device_ids:
            ids = (ctypes.c_int64 * len(device_ids))(*device_ids)
            rc = lib.axon_start_nrt_profile(ids, len(device_ids))
        else:
            rc = lib.axon_start_nrt_profile(None, 0)
        if rc != 0:
            raise RuntimeError(f"axon_start_nrt_profile rc={rc}")
        try:
            yield
        finally:
            n = lib.axon_stop_nrt_profile(str(output_dir).encode())
            print(f"profile: {n} file(s) written to {output_dir}", file=sys.stderr)

    mod._hook = _hook


_install_ntff_hook()

_CACHE = {}


def _get_nc():
    if "nc" not in _CACHE:
        nc = build_nc()
        nc.finalize()
        _CACHE["nc"] = nc
    return _CACHE["nc"]


def make_in_maps(ego_fut_preds, lane_preds, lane_score_preds):
    ego_fut_preds = np.ascontiguousarray(ego_fut_preds, dtype=np.float32)
    lane_preds = np.ascontiguousarray(lane_preds, dtype=np.float32)
    lane_score_preds = np.ascontiguousarray(lane_score_preds, dtype=np.float32)
    in_maps = []
    for c in range(NCORES):
        s = slice(c * BL, (c + 1) * BL)
        in_maps.append(
            {
                "lane": np.ascontiguousarray(lane_preds[s]),
                "score": np.ascontiguousarray(lane_score_preds[s]),
                "ego": np.ascontiguousarray(ego_fut_preds[s]),
            }
        )
    return in_maps


def run_on_hw(ego_fut_preds, lane_preds, lane_score_preds, trace=False):
    from concourse.bass_utils import run_bass_kernel_spmd

    nc = _get_nc()
    in_maps = make_in_maps(ego_fut_preds, lane_preds, lane_score_preds)
    res = run_bass_kernel_spmd(nc, in_maps, list(range(NCORES)), trace=trace)
    return res


def kernel(ego_fut_preds, lane_preds, lane_score_preds):
    res = run_on_hw(ego_fut_preds, lane_preds, lane_score_preds, trace=False)
    total = 0.0
    for r in res.results:
        total += np.asarray(r["loss"], dtype=np.float64).sum()
    return np.float32(total / (B * T))


# revision 24
# speedup vs baseline: 1.2089x; 1.2089x over previous
"""Trainium2 Bass kernel for nn_PlanMapBoundLoss.

Strategy (pure data parallel, one batch per SBUF partition):
  - 8 cores x 128 batches each (B=1024).
  - Per core: lane polylines live as [128, 4000] f32 in SBUF (xy interleaved).
  - Per timestep t (6): squared distances to all 2000 lane points are computed
    with two ScalarE Square activations (denorm fused in: Square(scale*x+bias)
    with per-partition bias = -offset - pred_t), summed on VectorE, then a
    grouped min-reduce (negated) gives per-lane -min dist^2 [128, 100].
    A -1e30 penalty masks non-boundary lanes; Max8 + MaxIndex give the
    nearest-lane id and -min dist^2.
  - The nearest lane's 20 points are fetched per-batch with an indirect DMA
    gather from HBM (row = (b, v), 160B).
  - Segment intersection (ego segment t vs 19 lane segments) is evaluated
    division-free on VectorE at [128, 6, 19]; its cummax over t zeroes the
    loss suffix. Per-batch losses [128, 6] are DMA'd out; the host sums.
"""

import sys

if "/opt/trn_rl_repo" not in sys.path:
    sys.path.insert(0, "/opt/trn_rl_repo")

from contextlib import ExitStack

import numpy as np

import concourse.bacc as bacc
import concourse.bass as bass
import concourse.tile as tile
from concourse import mybir
from concourse.bass import AP

F32 = mybir.dt.float32
U32 = mybir.dt.uint32
Alu = mybir.AluOpType
Act = mybir.ActivationFunctionType
Axis = mybir.AxisListType

B, T, V, P, C = 1024, 6, 100, 20, 3
NCORES = 8
BL = B // NCORES  # 128 batches per core == partition count
J = V * P  # 2000 lane points per batch
SCALE_X, OFF_X = 30.0, -15.0  # pc_range denorm
SCALE_Y, OFF_Y = 60.0, -30.0
MAP_THRESH = 0.5
DIS_THRESH = 1.0
PEN = -1.0e30

NJCHUNK = 2  # split the 2000-wide work for DMA/compute overlap
JC = J // NJCHUNK  # 1000 points per chunk
VC = V // NJCHUNK  # 50 lanes per chunk


def _bcast_last(ap: AP, n: int) -> AP:
    """Append a stride-0 broadcast dim of size n to an AP."""
    return AP(tensor=ap.tensor, offset=ap.offset, ap=ap.ap + [[0, n]])


def build_nc(debug_taps=False):
    nc = bacc.Bacc(None, target_bir_lowering=False)

    lane = nc.dram_tensor("lane", [BL, V, P, 2], F32, kind="ExternalInput")
    score = nc.dram_tensor("score", [BL, V, C], F32, kind="ExternalInput")
    ego = nc.dram_tensor("ego", [BL, T, 2], F32, kind="ExternalInput")
    out = nc.dram_tensor("loss", [BL, T], F32, kind="ExternalOutput")

    lane_flat = lane[:].rearrange("b v p c -> b (v p c)")  # [128, 4000]
    lane_rows = lane[:].rearrange("b v p c -> (b v) (p c)")  # [12800, 40]

    with ExitStack() as ctx:
        tc = ctx.enter_context(tile.TileContext(nc))
        singles = ctx.enter_context(tc.tile_pool(name="singles", bufs=1))
        work = ctx.enter_context(tc.tile_pool(name="work", bufs=2))

        # ---- loads: small tensors on the gpsimd queue, lane bulk on sync ----
        score_sb = singles.tile([BL, V * C], F32)
        nc.gpsimd.dma_start(out=score_sb[:], in_=score[:].rearrange("b v c -> b (v c)"))
        ego_sb = singles.tile([BL, T * 2], F32)
        nc.gpsimd.dma_start(out=ego_sb[:], in_=ego[:].rearrange("b t c -> b (t c)"))
        lane_sb = singles.tile([BL, 2 * J], F32)  # xy interleaved
        for c in range(NJCHUNK):
            nc.sync.dma_start(
                out=lane_sb[:, c * 2 * JC : (c + 1) * 2 * JC],
                in_=lane_flat[:, c * 2 * JC : (c + 1) * 2 * JC],
            )
        # dummy Square: hoist the ACT table load off the critical path
        dumm = singles.tile([BL, 8], F32)
        nc.vector.memset(dumm[:], 0.0)
        nc.scalar.activation(out=dumm[:], in_=dumm[:], func=Act.Square, bias=0.0, scale=1.0)

        lane3 = lane_sb[:].rearrange("b (j c) -> b j c", c=2)
        lane_x = lane3[:, :, 0]  # [128, 2000] stride-2 view
        lane_y = lane3[:, :, 1]

        # ---- penalty for non-boundary lanes ----
        pen = singles.tile([BL, V], F32)
        score3 = score_sb[:].rearrange("b (v c) -> b v c", c=C)
        nc.vector.tensor_scalar(
            out=pen[:],
            in0=score3[:, :, 2],
            scalar1=MAP_THRESH,
            scalar2=PEN,
            op0=Alu.is_lt,
            op1=Alu.mult,
        )

        # ---- trajectory cumsum and per-t scalars ----
        ego3 = ego_sb[:].rearrange("b (t c) -> b t c", c=2)
        ego_x = ego3[:, :, 0]  # [128, 6] stride-2
        ego_y = ego3[:, :, 1]
        zeros6 = singles.tile([BL, T], F32)
        nc.vector.memset(zeros6[:], 0.0)
        px = singles.tile([BL, T], F32)
        py = singles.tile([BL, T], F32)
        nc.vector.tensor_tensor_scan(
            out=px[:], data0=ego_x, data1=zeros6[:], initial=0.0,
            op0=Alu.add, op1=Alu.add,
        )
        nc.vector.tensor_tensor_scan(
            out=py[:], data0=ego_y, data1=zeros6[:], initial=0.0,
            op0=Alu.add, op1=Alu.add,
        )
        # biases for the Square activation: -OFF - pred
        biasx = singles.tile([BL, T], F32)
        biasy = singles.tile([BL, T], F32)
        nc.vector.tensor_scalar(
            out=biasx[:], in0=px[:], scalar1=-1.0, scalar2=OFF_X,
            op0=Alu.mult, op1=Alu.add,
        )
        nc.vector.tensor_scalar(
            out=biasy[:], in0=py[:], scalar1=-1.0, scalar2=OFF_Y,
            op0=Alu.mult, op1=Alu.add,
        )
        # segment starts: s1[t] = pred[t-1], s1[0] = 0
        s1x = singles.tile([BL, T], F32)
        s1y = singles.tile([BL, T], F32)
        nc.vector.memset(s1x[:, 0:1], 0.0)
        nc.vector.memset(s1y[:, 0:1], 0.0)
        nc.vector.tensor_copy(s1x[:, 1:T], px[:, 0 : T - 1])
        nc.vector.tensor_copy(s1y[:, 1:T], py[:, 0 : T - 1])
        # d1[t] = pred[t] - s1[t] (match reference rounding)
        d1x = singles.tile([BL, T], F32)
        d1y = singles.tile([BL, T], F32)
        nc.vector.tensor_tensor(out=d1x[:], in0=px[:], in1=s1x[:], op=Alu.subtract)
        nc.vector.tensor_tensor(out=d1y[:], in0=py[:], in1=s1y[:], op=Alu.subtract)

        # row index base for the gather: 100*b
        iotap = singles.tile([BL, 1], U32)
        nc.gpsimd.iota(iotap[:], pattern=[[0, 1]], base=0, channel_multiplier=V)

        # ---- persistent per-t outputs ----
        top8all = singles.tile([BL, T * 8], F32)
        idx8all = singles.tile([BL, T * 8], U32)
        rowidx = singles.tile([BL, T], U32)
        TH = T // 2
        # two gather tiles so the first intersection half only waits on t0-2
        gathA = singles.tile([BL, TH * P * 2], F32)
        gathB = singles.tile([BL, TH * P * 2], F32)

        # ---- main t loop ----
        for t in range(T):
            sqx = work.tile([BL, J], F32, tag="sqx")
            sqy = work.tile([BL, J], F32, tag="sqy")
            ss = work.tile([BL, J], F32, tag="ss")
            dmn = work.tile([BL, V], F32, tag="dmn")
            dm2 = work.tile([BL, V], F32, tag="dm2")
            if t == 0:
                # chunked: lets the first squares start on the first DMA chunk
                for c in range(NJCHUNK):
                    js = slice(c * JC, (c + 1) * JC)
                    nc.scalar.activation(
                        out=sqx[:, js], in_=lane_x[:, js], func=Act.Square,
                        bias=biasx[:, t : t + 1], scale=SCALE_X,
                    )
                    nc.scalar.activation(
                        out=sqy[:, js], in_=lane_y[:, js], func=Act.Square,
                        bias=biasy[:, t : t + 1], scale=SCALE_Y,
                    )
            else:
                nc.scalar.activation(
                    out=sqx[:], in_=lane_x, func=Act.Square,
                    bias=biasx[:, t : t + 1], scale=SCALE_X,
                )
                nc.scalar.activation(
                    out=sqy[:], in_=lane_y, func=Act.Square,
                    bias=biasy[:, t : t + 1], scale=SCALE_Y,
                )
            # split the adds: first half on DVE, second half on GpSimd
            nc.vector.tensor_tensor(
                out=ss[:, 0:JC], in0=sqx[:, 0:JC], in1=sqy[:, 0:JC], op=Alu.add
            )
            nc.gpsimd.tensor_tensor(
                out=ss[:, JC:J], in0=sqx[:, JC:J], in1=sqy[:, JC:J], op=Alu.add
            )
            ss3 = ss[:].rearrange("b (v p) -> b v p", p=P)
            nc.vector.tensor_reduce(
                out=dmn[:], in_=ss3, op=Alu.min, axis=Axis.X, negate=True,
            )
            nc.vector.tensor_tensor(out=dm2[:], in0=dmn[:], in1=pen[:], op=Alu.add)
            t8 = top8all[:, 8 * t : 8 * t + 8]
            i8 = idx8all[:, 8 * t : 8 * t + 8]
            nc.vector.max(t8, dm2[:])
            nc.vector.max_index(i8, t8, dm2[:])
            nc.vector.tensor_tensor(
                out=rowidx[:, t : t + 1], in0=iotap[:], in1=i8[:, 0:1], op=Alu.add
            )
            gtile = gathA if t < TH else gathB
            toff = t if t < TH else t - TH
            nc.gpsimd.indirect_dma_start(
                out=gtile[:, 2 * P * toff : 2 * P * (toff + 1)],
                out_offset=None,
                in_=lane_rows,
                in_offset=bass.IndirectOffsetOnAxis(ap=rowidx[:, t : t + 1], axis=0),
            )
            if debug_taps and t == 0:
                for nm, tl in (("d_sqx", sqx), ("d_sqy", sqy), ("d_ss", ss), ("d_dmn", dmn), ("d_dm2", dm2)):
                    dbg = nc.dram_tensor(nm, list(tl.shape), F32, kind="ExternalOutput")
                    nc.sync.dma_start(out=dbg[:], in_=tl[:])

        # ---- min distance -> loss ----
        t8v = top8all[:].rearrange("b (t e) -> b t e", e=8)
        mind = singles.tile([BL, T], F32)
        nc.scalar.activation(out=mind[:], in_=t8v[:, :, 0], func=Act.Sqrt, scale=-1.0)
        loss6 = singles.tile([BL, T], F32)
        nc.scalar.activation(
            out=loss6[:], in_=mind[:], func=Act.Relu, scale=-1.0, bias=DIS_THRESH
        )

        # ---- segment intersection, two t-halves (first overlaps later gathers) ----
        S = P - 1  # 19 segments
        inter = singles.tile([BL, T], F32)

        for h in range(2):
            ts0 = h * TH
            tsl = slice(ts0, ts0 + TH)
            gsrc = gathA if h == 0 else gathB
            g3 = gsrc[:].rearrange("b (t p c) -> b t p c", p=P, c=2)
            gx = work.tile([BL, TH * P], F32, tag=f"gx{h}", name=f"gx{h}")
            gy = work.tile([BL, TH * P], F32, tag=f"gy{h}", name=f"gy{h}")
            gx3 = gx[:].rearrange("b (t p) -> b t p", p=P)
            gy3 = gy[:].rearrange("b (t p) -> b t p", p=P)
            nc.scalar.activation(
                out=gx3, in_=g3[:, :, :, 0], func=Act.Copy, scale=SCALE_X, bias=OFF_X
            )
            nc.scalar.activation(
                out=gy3, in_=g3[:, :, :, 1], func=Act.Copy, scale=SCALE_Y, bias=OFF_Y
            )
            W = TH * S

            def wt(name_tag):
                tl = work.tile([BL, W], F32, tag=name_tag, name=name_tag)
                return tl[:].rearrange("b (t s) -> b t s", s=S)

            s2x = gx3[:, :, 0:S]
            s2y = gy3[:, :, 0:S]
            e2x = gx3[:, :, 1 : S + 1]
            e2y = gy3[:, :, 1 : S + 1]
            d2x = wt(f"d2x{h}")
            d2y = wt(f"d2y{h}")
            nc.vector.tensor_tensor(out=d2x, in0=e2x, in1=s2x, op=Alu.subtract)
            nc.vector.tensor_tensor(out=d2y, in0=e2y, in1=s2y, op=Alu.subtract)
            dlx = wt(f"dlx{h}")
            dly = wt(f"dly{h}")
            nc.vector.tensor_tensor(
                out=dlx, in0=s2x, in1=_bcast_last(s1x[:, tsl], S), op=Alu.subtract
            )
            nc.vector.tensor_tensor(
                out=dly, in0=s2y, in1=_bcast_last(s1y[:, tsl], S), op=Alu.subtract
            )
            # det = d1x*d2y - d1y*d2x
            ta = wt(f"ta{h}")
            tb = wt(f"tb{h}")
            det = wt(f"det{h}")
            nc.vector.tensor_tensor(out=ta, in0=d2y, in1=_bcast_last(d1x[:, tsl], S), op=Alu.mult)
            nc.vector.tensor_tensor(out=tb, in0=d2x, in1=_bcast_last(d1y[:, tsl], S), op=Alu.mult)
            nc.vector.tensor_tensor(out=det, in0=ta, in1=tb, op=Alu.subtract)
            # t_num = dlx*d2y - dly*d2x
            tnum = wt(f"tnum{h}")
            nc.vector.tensor_tensor(out=ta, in0=dlx, in1=d2y, op=Alu.mult)
            nc.vector.tensor_tensor(out=tb, in0=dly, in1=d2x, op=Alu.mult)
            nc.vector.tensor_tensor(out=tnum, in0=ta, in1=tb, op=Alu.subtract)
            # u_num = dlx*d1y - dly*d1x  (u-chain on GpSimd, parallel with t-chain)
            unum = wt(f"unum{h}")
            ua = wt(f"ua{h}")
            ub = wt(f"ub{h}")
            nc.vector.tensor_tensor(
                out=ua, in0=dlx, in1=_bcast_last(d1y[:, tsl], S), op=Alu.mult
            )
            nc.vector.tensor_tensor(
                out=ub, in0=dly, in1=_bcast_last(d1x[:, tsl], S), op=Alu.mult
            )
            nc.vector.tensor_tensor(out=unum, in0=ua, in1=ub, op=Alu.subtract)
            # conditions: t in [0,1], u in [0,1], det != 0 (division-free, exact)
            cmin = wt(f"cmin{h}")
            nc.vector.tensor_tensor(out=ta, in0=tnum, in1=det, op=Alu.mult)  # t>=0
            nc.vector.tensor_tensor(out=tb, in0=det, in1=tnum, op=Alu.subtract)
            nc.vector.tensor_tensor(out=tb, in0=tb, in1=det, op=Alu.mult)  # t<=1
            nc.vector.tensor_tensor(out=cmin, in0=ta, in1=tb, op=Alu.min)
            nc.vector.tensor_tensor(out=ua, in0=unum, in1=det, op=Alu.mult)  # u>=0
            nc.vector.tensor_tensor(out=ub, in0=det, in1=unum, op=Alu.subtract)
            nc.vector.tensor_tensor(out=ub, in0=ub, in1=det, op=Alu.mult)  # u<=1
            nc.vector.tensor_tensor(out=ua, in0=ua, in1=ub, op=Alu.min)
            nc.vector.tensor_tensor(out=cmin, in0=cmin, in1=ua, op=Alu.min)
            # inter01 = (cmin >= 0) * (det != 0)
            nc.vector.tensor_scalar(
                out=cmin, in0=cmin, scalar1=0.0, scalar2=None, op0=Alu.is_ge
            )
            nc.vector.tensor_scalar(
                out=ta, in0=det, scalar1=0.0, scalar2=None, op0=Alu.not_equal
            )
            nc.vector.tensor_tensor(out=cmin, in0=cmin, in1=ta, op=Alu.mult)
            nc.vector.tensor_reduce(out=inter[:, tsl], in_=cmin, op=Alu.max, axis=Axis.X)

        # ---- zero the suffix from the first intersecting t ----
        cm = singles.tile([BL, T], F32)
        nc.vector.tensor_tensor_scan(
            out=cm[:], data0=inter[:], data1=zeros6[:], initial=0.0,
            op0=Alu.max, op1=Alu.add,
        )
        keep = singles.tile([BL, T], F32)
        nc.vector.tensor_scalar(
            out=keep[:], in0=cm[:], scalar1=-1.0, scalar2=1.0,
            op0=Alu.mult, op1=Alu.add,
        )
        lf = singles.tile([BL, T], F32)
        nc.vector.tensor_tensor(out=lf[:], in0=loss6[:], in1=keep[:], op=Alu.mult)
        nc.sync.dma_start(out=out[:], in_=lf[:])

        if debug_taps:
            taps = {
                "d_px": px, "d_biasx": biasx, "d_pen": pen, "d_top8": top8all,
                "d_idx8": idx8all, "d_rowidx": rowidx,
                "d_mind": mind, "d_loss6": loss6, "d_inter": inter,
                "d_cm": cm, "d_keep": keep,
            }
            for nm, tl in taps.items():
                dt_ = tl.tensor.dtype if hasattr(tl, "tensor") else F32
                dbg = nc.dram_tensor(nm, list(tl.shape), dt_, kind="ExternalOutput")
                nc.sync.dma_start(out=dbg[:], in_=tl[:])

    return nc


def _install_ntff_hook():
    """Provide antenv.axon_hooks (missing in this image) so trace=True works.

    Replicates trn_agent_boot._ntff_profile_via_ctypes against the axon PJRT
    .so: start/stop NRT profiling around the execute, NTFFs land in the
    requested output dir.
    """
    import sys as _sys
    import types
    import contextlib
    import ctypes

    if "antenv.axon_hooks" in _sys.modules:
        return
    try:
        import antenv
    except ImportError:
        return
    so_path = "/opt/axon/libaxon_pjrt.so"
    mod = types.ModuleType("antenv.axon_hooks")
    mod._hook = None

    def set_axon_ntff_profile_hook(h):
        mod._hook = h

    def get_axon_ntff_profile_hook():
        return mod._hook

    mod.set_axon_ntff_profile_hook = set_axon_ntff_profile_hook
    mod.get_axon_ntff_profile_hook = get_axon_ntff_profile_hook
    _sys.modules["antenv.axon_hooks"] = mod
    antenv.axon_hooks = mod

    try:
        lib = ctypes.CDLL(so_path)
    except OSError:
        return
    if not hasattr(lib, "axon_start_nrt_profile"):
        return
    lib.axon_start_nrt_profile.argtypes = [
        ctypes.POINTER(ctypes.c_int64),
        ctypes.c_size_t,
    ]
    lib.axon_start_nrt_profile.restype = ctypes.c_int64
    lib.axon_stop_nrt_profile.argtypes = [ctypes.c_char_p]
    lib.axon_stop_nrt_profile.restype = ctypes.c_int64

    @contextlib.contextmanager
    def _hook(output_dir, device_ids):
        import jax

        jax.devices()
        if device_ids:
            ids = (ctypes.c_int64 * len(device_ids))(*device_ids)
            rc = lib.axon_start_nrt_profile(ids, len(device_ids))
        else:
            rc = lib.axon_start_nrt_profile(None, 0)
        if rc != 0:
            raise RuntimeError(f"axon_start_nrt_profile rc={rc}")
        try:
            yield
        finally:
            n = lib.axon_stop_nrt_profile(str(output_dir).encode())
            print(f"profile: {n} file(s) written to {output_dir}", file=sys.stderr)

    mod._hook = _hook


_install_ntff_hook()

_CACHE = {}


def _get_nc():
    if "nc" not in _CACHE:
        nc = build_nc()
        nc.finalize()
        _CACHE["nc"] = nc
    return _CACHE["nc"]


def make_in_maps(ego_fut_preds, lane_preds, lane_score_preds):
    ego_fut_preds = np.ascontiguousarray(ego_fut_preds, dtype=np.float32)
    lane_preds = np.ascontiguousarray(lane_preds, dtype=np.float32)
    lane_score_preds = np.ascontiguousarray(lane_score_preds, dtype=np.float32)
    in_maps = []
    for c in range(NCORES):
        s = slice(c * BL, (c + 1) * BL)
        in_maps.append(
            {
                "lane": np.ascontiguousarray(lane_preds[s]),
                "score": np.ascontiguousarray(lane_score_preds[s]),
                "ego": np.ascontiguousarray(ego_fut_preds[s]),
            }
        )
    return in_maps


def run_on_hw(ego_fut_preds, lane_preds, lane_score_preds, trace=False):
    from concourse.bass_utils import run_bass_kernel_spmd

    nc = _get_nc()
    in_maps = make_in_maps(ego_fut_preds, lane_preds, lane_score_preds)
    res = run_bass_kernel_spmd(nc, in_maps, list(range(NCORES)), trace=trace)
    return res


def kernel(ego_fut_preds, lane_preds, lane_score_preds):
    res = run_on_hw(ego_fut_preds, lane_preds, lane_score_preds, trace=False)
    total = 0.0
    for r in res.results:
        total += np.asarray(r["loss"], dtype=np.float64).sum()
    return np.array(total / (B * T), dtype=np.float32)
